# revision 13
# baseline (speedup 1.0000x reference)
"""Trainium2 Bass kernel for nn_AttentionSpatial (manifold attention), v4.

Pipeline (per the reference):
  qkv = 1x1 conv -> 3x3 depthwise conv -> patchify -> per-(b,head,c) unit:
  normalize q,k -> attn = softmax(q k^T * temp) -> cov -> eigh (top-100)
  -> A = U (w_fr^T w_fr) U^T -> out = A v -> re-patchify -> 1x1 conv out.

Sharding: 32 units = (batch 2) x (channel 16); core k=(b,g) owns channels
4g..4g+4 of batch b.  Eigh runs on host (jax CPU f64) because the final
output depends on LAPACK eigenvector signs; everything else on device.

v4 changes vs v2 baseline:
  - conv in 3-pass f16 split precision (x=xh+xl, w=wh+wl; passes
    wh*xh + wh*xl + (wl*16)*(xh/16)); ~5e-7 rel error (vs f32 1e-7),
    3 cyc/row instead of f32's 4, and each pass streams at 1 cyc/row.
  - patchify moved fully on-chip: two PE-transpose stages image->np-major
    (device patch order n_dev = pb*128 + band*16 + pc; host unpermutes).
    Kills the DRAM round-trip + 64B-line gather DMAs.
  - v ships as np-major vpd directly (no host patchify).
  - p3 folded into p2: each core emits the 16-channel partial projection
    partial[o] = sum_u w_po[o,c_u] m_u (bf16); host sums 4 partials.
  - attn/cov matmuls stay f32 (eigh amplifies cov error ~1000x; f32r's
    1.5e-4 matmul error gives 0.2 final rel err - measured).
"""

import numpy as np

PATCH = 16
HEADS = 4
TOP_P = 100
B, C = 2, 16
NCORES = 8
S_SPLIT = 16.0

_built = {}
PROFILE = False
LAST_PROFILE = []
DEBUG = {}

# device patch order: n_dev = pb*128 + band*16 + pc  <->  nat = (2*band+pb)*16+pc
_i = np.arange(256)
P_NAT = ((2 * ((_i >> 4) & 7) + (_i >> 7)) * 16 + (_i & 15)).astype(np.int64)
P_INV = np.zeros(256, np.int64)
P_INV[P_NAT] = _i


def _new_nc():
    from concourse import bacc
    return bacc.Bacc("TRN2", target_bir_lowering=False, debug=False)


# --------------------------------------------------------------------------
# L1: 3-pass f16 conv + on-chip patchify + attention + cov
# --------------------------------------------------------------------------

def _build_p1():
    import concourse.bass as bass
    import concourse.tile as tile
    from concourse import mybir
    from concourse.masks import make_identity

    f32 = mybir.dt.float32
    f16 = mybir.dt.float16
    AF = mybir.ActivationFunctionType
    OP = mybir.AluOpType
    nc = _new_nc()

    xh_d = nc.dram_tensor("xh", (16, 258, 258), f16, kind="ExternalInput")
    xl_d = nc.dram_tensor("xl", (16, 258, 258), f16, kind="ExternalInput")
    xs_d = nc.dram_tensor("xs", (16, 258, 258), f16, kind="ExternalInput")
    wh_d = nc.dram_tensor("wh", (128, 9 * 96), f16, kind="ExternalInput")
    wl_d = nc.dram_tensor("wl", (128, 9 * 96), f16, kind="ExternalInput")
    tmp_d = nc.dram_tensor("tempu", (1, 1), f32, kind="ExternalInput")
    vpd_d = nc.dram_tensor("vpd", (4, 128, 512), f32, kind="ExternalOutput")
    cov_d = nc.dram_tensor("cov", (4, 256, 256), f32, kind="ExternalOutput")

    with tile.TileContext(nc) as tc:
        with (
            tc.tile_pool(name="big", bufs=1) as big,
            tc.tile_pool(name="pdp", bufs=1) as pdp,
            tc.tile_pool(name="unit", bufs=4) as up,
            tc.tile_pool(name="small", bufs=3) as sp,
            tc.tile_pool(name="psC", bufs=2, space="PSUM") as psC,
            tc.tile_pool(name="psTT", bufs=2, space="PSUM") as psTT,
            tc.tile_pool(name="psT", bufs=2, space="PSUM") as psT,
            tc.tile_pool(name="psA", bufs=2, space="PSUM") as psA,
        ):
            # ---- x in four chained slabs per tensor (xh/xl/xs on separate
            # queues); marker copies serialize each chain so slab 0 lands
            # at full bandwidth.
            SLABS = ((0, 8), (6, 16), (14, 24), (22, 34))
            xqs = {}
            for name, dram, eng in (("xh", xh_d, nc.sync),
                                    ("xl", xl_d, nc.scalar),
                                    ("xs", xs_d, nc.gpsimd)):
                tiles = []
                for q, (r0, r1) in enumerate(SLABS):
                    t_xq = big.tile([128, r1 - r0, 258], f16,
                                    tag=f"{name}{q}")
                    tiles.append(t_xq)
                for q, (r0, r1) in enumerate(SLABS):
                    if q > 0:
                        nc.vector.tensor_copy(tiles[q][0:1, 0, 0:1],
                                              tiles[q - 1][0:1, 0, 0:1])
                    eng.dma_start(
                        tiles[q],
                        bass.AP(tensor=dram.ap().tensor, offset=r0 * 258,
                                ap=[[32 * 258, 8], [258 * 258, 16],
                                    [1, (r1 - r0) * 258]]),
                    )
                xqs[name] = tiles

            wh = big.tile([128, 9, 96], f16, tag="wh")
            nc.sync.dma_start(wh.rearrange("p a b -> p (a b)"), wh_d.ap())
            wl = big.tile([128, 9, 96], f16, tag="wl")
            nc.scalar.dma_start(wl.rearrange("p a b -> p (a b)"), wl_d.ap())

            ident = big.tile([128, 128], f32, tag="ident")
            make_identity(nc, ident)
            tempb = big.tile([128, 1], f32, tag="tempb")
            nc.sync.dma_start(
                tempb,
                bass.AP(tensor=tmp_d.ap().tensor, offset=0,
                        ap=[[0, 128], [1, 1]]),
            )

            # ---- HAM warm-up: dense bf16 matmuls spanning the head so the
            # PE clock is ramped when the conv starts
            identb = big.tile([128, 128], mybir.dt.bfloat16, tag="identb")
            nc.vector.tensor_copy(identb, ident)
            junk = big.tile([128, 512], mybir.dt.bfloat16, tag="junk")
            nc.vector.memset(junk, 1.0)
            prm = psT.tile([128, 2, 256], f32, tag="tps")
            for i in range(24):
                nc.tensor.matmul(prm.rearrange("p a b -> p (a b)"), identb,
                                 junk, start=(i == 0), stop=(i == 23))

            # ---- conv: 32 chunks (one image row across 8 bands), 27 f16
            # matmuls each (9 taps x 3 split passes)
            # q2all[(o,band), pb, pc, dc0, dr, dcH]: T1 reads the
            # (dr,dcH) 128-block contiguously; the conv copy absorbs the
            # column permute
            q2all = big.tile([96, 2, 16, 2, 16, 8], f32, tag="q2all")
            taps = [(dy, dx) for dy in (-1, 0, 1) for dx in (-1, 0, 1)]
            for r in range(32):
                acc = psC.tile([96, 256], f32, tag="conv")
                q = 0 if r < 6 else (1 if r < 14 else (2 if r < 22 else 3))
                r0 = SLABS[q][0]
                k = 0
                for t, (dy, dx) in enumerate(taps):
                    row = r - r0 + 1 + dy
                    cs = slice(dx + 1, dx + 257)
                    for wt_, xn in ((wh, "xh"), (wh, "xl"), (wl, "xs")):
                        nc.tensor.matmul(acc, wt_[:, t, :],
                                         xqs[xn][q][:, row, cs],
                                         start=(k == 0), stop=(k == 26))
                        k += 1
                dst = q2all[:, r // 16, :, :, r % 16, :]
                src_v = acc.rearrange("p (a b c) -> p a c b", a=16, b=8)
                if r % 2 == 0:
                    nc.vector.tensor_copy(dst, src_v)
                else:
                    nc.scalar.copy(dst, src_v)

            # ---- T1: [(o,band), (dr,dcH)-slice] -> [(dr,dcH), (o,band)]
            # per (pb, pc, dc0); wave pb=0 issued mid-conv order-wise
            mid = big.tile([128, 2, 12, 2, 8, 16], f32, tag="mid")
            # gpsimd cannot access PSUM; alternate vector/scalar
            CPY = [nc.vector.tensor_copy, nc.scalar.copy]

            def t1_wave(pb):
                for i, (pc, dc0) in enumerate(
                        (pc, dc0) for pc in range(16) for dc0 in range(2)):
                    tps = psTT.tile([128, 128], f32, tag="tt")
                    nc.tensor.transpose(
                        tps[:, 0:96],
                        q2all[:, pb, pc, dc0].rearrange("p a b -> p (a b)"),
                        ident[0:96, 0:96])
                    CPY[i % 2](
                        mid[:, pb, :, dc0, :, pc],
                        tps[:, 0:96].rearrange("p (o bd) -> p o bd", o=12))

            # ---- T2: [(dr,dcH), (band,pc)] -> np-major pd per channel
            pd = {}
            for o in range(12):
                t_pd = pdp.tile([128, 2, 16, 8, 2], f32, tag=f"pd{o}")
                pd[o] = t_pd

            def t2_wave(pb, o_list):
                for i, (o, dc0) in enumerate(
                        (o, dc0) for o in o_list for dc0 in range(2)):
                    tps = psTT.tile([128, 128], f32, tag="tt")
                    nc.tensor.transpose(
                        tps, mid[:, pb, o, dc0].rearrange("p a b -> p (a b)"),
                        ident)
                    CPY[i % 2](
                        pd[o][:, pb, :, :, dc0],
                        tps.rearrange("p (a b) -> p a b", a=16))

            t1_wave(0)
            t1_wave(1)
            t2_wave(0, range(12))
            t2_wave(1, range(12))

            def pdh(o, c2):
                return pd[o][:, c2].rearrange("p b c d -> p (b c d)")

            # ---- v out (np-major, contiguous)
            for o in range(8, 12):
                eng = (nc.sync, nc.scalar, nc.gpsimd, nc.sync)[o - 8]
                eng.dma_start(
                    vpd_d.ap()[o - 8],
                    pd[o].rearrange("p a b c d -> p (a b c d)"))

            # ---- normalize q,k rows (np-major; norm over free=d)
            def normalize(o):
                scr = sp.tile([128, 256], f32, tag="scr")
                nrm2 = sp.tile([128, 2], f32, tag="nrm2")
                for c2 in range(2):
                    nc.scalar.activation(scr, pdh(o, c2), AF.Square,
                                         accum_out=nrm2[:, c2:c2 + 1])
                nc.vector.tensor_scalar_max(nrm2, nrm2, 1e-24)
                srt = sp.tile([128, 2], f32, tag="srt")
                nc.scalar.sqrt(srt, nrm2)
                rin = sp.tile([128, 2], f32, tag="rin")
                nc.vector.reciprocal(rin, srt)
                # one newton step on rsqrt fixes table-sqrt error:
                # r1 = r0*(1.5 - 0.5*n2*r0^2)
                nwt = sp.tile([128, 2], f32, tag="nwt")
                nc.vector.tensor_mul(nwt, rin, rin)
                nc.vector.tensor_mul(nwt, nwt, nrm2)
                nc.vector.tensor_scalar(nwt, nwt, -0.5, 1.5,
                                        op0=OP.mult, op1=OP.add)
                nc.vector.tensor_mul(rin, rin, nwt)
                for c2 in range(2):
                    nc.vector.tensor_scalar_mul(
                        pdh(o, c2), pdh(o, c2), rin[:, c2:c2 + 1])

            # ---- per unit: normalize + q/k transposes (T3) interleaved
            qTs, kTs, att_es, rssums, xcs, xcTs = {}, {}, {}, {}, {}, {}
            for u in range(4):
                normalize(u)
                normalize(4 + u)
                qT = up.tile([128, 2, 256], f32, tag="qT")
                qTs[u] = qT
                kT = up.tile([128, 2, 256], f32, tag="kT")
                kTs[u] = kT
                for ti, (src_o, dst_t) in enumerate(((u, qT), (4 + u, kT))):
                    tps = psT.tile([128, 2, 256], f32, tag="tps")
                    for nh in range(2):
                        for dh in range(2):
                            nc.tensor.transpose(
                                tps[:, dh, 128 * nh:128 * (nh + 1)],
                                pd[src_o][:, nh, 8 * dh:8 * (dh + 1)]
                                .rearrange("p a b c -> p (a b c)"),
                                ident)
                    if ti == 0:
                        nc.vector.tensor_copy(dst_t, tps)
                    else:
                        nc.scalar.copy(dst_t, tps)

            for u in range(4):
                att_e = up.tile([128, 2, 256], f32, tag="att_e")
                att_es[u] = att_e
                rssum = sp.tile([128, 2], f32, tag=f"rssum{u}")
                rssums[u] = rssum
                for nh in range(2):
                    att = psA.tile([128, 256], f32, tag="att")
                    for dh in range(2):
                        nc.tensor.matmul(
                            att, qTs[u][:, dh, 128 * nh:128 * (nh + 1)],
                            kTs[u][:, dh, :], start=(dh == 0),
                            stop=(dh == 1))
                    nc.scalar.activation(att_e[:, nh, :], att, AF.Exp,
                                         scale=tempb[:, 0:1],
                                         accum_out=rssum[:, nh:nh + 1])

            for u in range(4):
                rinv = sp.tile([128, 2], f32, tag=f"rinv{u}")
                nc.vector.reciprocal(rinv, rssums[u])
                xc = up.tile([128, 2, 256], f32, tag="xc")
                xcs[u] = xc
                for nh in range(2):
                    nc.vector.tensor_scalar(
                        xc[:, nh, :], att_es[u][:, nh, :],
                        rinv[:, nh:nh + 1], 1.0 / 256.0,
                        op0=OP.mult, op1=OP.subtract)

            for u in range(4):
                xcT = up.tile([128, 2, 256], f32, tag="xcT")
                xcTs[u] = xcT
                tps2 = psT.tile([128, 2, 256], f32, tag="tps")
                for nh in range(2):
                    for mh in range(2):
                        nc.tensor.transpose(
                            tps2[:, mh, 128 * nh:128 * (nh + 1)],
                            xcs[u][:, nh, 128 * mh:128 * (mh + 1)], ident)
                if u % 2 == 0:
                    nc.vector.tensor_copy(xcT, tps2)
                else:
                    nc.scalar.copy(xcT, tps2)

            for u in range(4):
                xcT = xcTs[u]
                cov_sb = up.tile([128, 2, 256], f32, tag="cov_sb")
                for nh in range(2):
                    cv = psA.tile([128, 256], f32, tag="att")
                    for mh in range(2):
                        nc.tensor.matmul(
                            cv, xcT[:, mh, 128 * nh:128 * (nh + 1)],
                            xcT[:, mh, :], start=(mh == 0), stop=(mh == 1))
                    if nh == 0:
                        nc.vector.tensor_copy(cov_sb[:, nh, :], cv)
                    else:
                        nc.scalar.copy(cov_sb[:, nh, :], cv)
                cov_view = cov_d.ap()[u].rearrange("(c p) m -> p c m",
                                                   p=128)
                for nh in range(2):
                    eng = nc.scalar if (2 * u + nh) % 2 == 0 else nc.sync
                    eng.dma_start(cov_view[:, nh, :], cov_sb[:, nh, :])

    nc.compile()
    return nc


# --------------------------------------------------------------------------
# L2: yT = w_fr U^T, A = Y Y^T, M = A v, partial = sum_u wpo[:,c_u] M_u
# --------------------------------------------------------------------------

def _build_p2():
    import concourse.tile as tile
    from concourse import mybir

    f32 = mybir.dt.float32
    f32r = mybir.dt.float32r
    bf16 = mybir.dt.bfloat16
    nc = _new_nc()

    ut_d = nc.dram_tensor("ut", (100, 1024), f32r, kind="ExternalInput")
    vpd_d = nc.dram_tensor("vpd", (4, 128, 512), f32r, kind="ExternalInput")
    wfrT_d = nc.dram_tensor("wfrT", (100, 100), f32r, kind="ExternalInput")
    wpoD_d = nc.dram_tensor("wpoD", (32, 128), bf16, kind="ExternalInput")
    po_d = nc.dram_tensor("po", (128, 8192), bf16, kind="ExternalOutput")

    with tile.TileContext(nc) as tc:
        with (
            tc.tile_pool(name="sb", bufs=1) as sb,
            tc.tile_pool(name="unit", bufs=4) as up,
            tc.tile_pool(name="ps", bufs=2, space="PSUM") as ps,
            tc.tile_pool(name="psP", bufs=2, space="PSUM") as psP,
        ):
            wfrT = sb.tile([100, 100], f32r, tag="wfrT")
            nc.gpsimd.dma_start(wfrT, wfrT_d.ap())
            wpoD = sb.tile([32, 128], bf16, tag="wpoD")
            nc.gpsimd.dma_start(wpoD, wpoD_d.ap())
            ut_all = sb.tile([100, 4, 256], f32r, tag="ut")
            nc.sync.dma_start(
                ut_all.rearrange("p a b -> p (a b)"), ut_d.ap())
            v_all = sb.tile([128, 4, 2, 256], f32r, tag="v")
            nc.scalar.dma_start(
                v_all, vpd_d.ap().rearrange("a p b -> p a b"))

            mpack = sb.tile([32, 8192], bf16, tag="mpack")
            DQ = [nc.sync, nc.scalar, nc.gpsimd]
            for u in range(4):
                yTp = ps.tile([100, 256], f32, tag="yT")
                nc.tensor.matmul(yTp, wfrT, ut_all[:, u, :],
                                 start=True, stop=True)
                yT = up.tile([100, 256], f32r, tag="yTs")
                nc.vector.tensor_copy(yT, yTp)

                a_sb = up.tile([128, 2, 256], f32r, tag="a_sb")
                for nh in range(2):
                    ap_ = ps.tile([128, 256], f32, tag="aps")
                    nc.tensor.matmul(ap_, yT[:, 128 * nh:128 * (nh + 1)],
                                     yT, start=True, stop=True)
                    if nh == 0:
                        nc.scalar.copy(a_sb[:, nh, :], ap_)
                    else:
                        nc.vector.tensor_copy(a_sb[:, nh, :], ap_)

                m_sb = up.tile([128, 2, 256], bf16, tag="m_sb")
                for mc in range(2):
                    mp = ps.tile([128, 256], f32, tag="mps")
                    for kc in range(2):
                        nc.tensor.matmul(
                            mp, a_sb[:, kc, 128 * mc:128 * (mc + 1)],
                            v_all[:, u, kc, :], start=(kc == 0),
                            stop=(kc == 1))
                    if mc == 0:
                        nc.scalar.copy(m_sb[:, mc, :], mp)
                    else:
                        nc.vector.tensor_copy(m_sb[:, mc, :], mp)
                # pack m_u rows into partitions (sp*4+u) for the
                # channel-mix contraction
                for s in range(8):
                    DQ[s % 3].dma_start(
                        mpack[4 * s + u:4 * s + u + 1, :],
                        m_sb[16 * s:16 * (s + 1)])

            # partial[o] = sum_u wpo[o,c_u] m_u  (block-diag over 8
            # partition groups)
            po_sb = sb.tile([128, 8192], bf16, tag="po_sb")
            for ch in range(16):
                pp = psP.tile([128, 512], f32, tag="pp")
                nc.tensor.matmul(pp, wpoD, mpack[:, 512 * ch:512 * (ch + 1)],
                                 start=True, stop=True)
                if ch % 2 == 0:
                    nc.vector.tensor_copy(
                        po_sb[:, 512 * ch:512 * (ch + 1)], pp)
                else:
                    nc.scalar.copy(po_sb[:, 512 * ch:512 * (ch + 1)], pp)
            for i in range(4):
                DQ[i % 3].dma_start(po_d.ap()[:, 2048 * i:2048 * (i + 1)],
                                    po_sb[:, 2048 * i:2048 * (i + 1)])

    nc.compile()
    return nc


# --------------------------------------------------------------------------
# host orchestration
# --------------------------------------------------------------------------

def _get(name):
    if name not in _built:
        _built[name] = {"p1": _build_p1, "p2": _build_p2}[name]()
    return _built[name]


def _run(name, nc, in_maps):
    from concourse.bass_utils import run_bass_kernel_spmd
    r = run_bass_kernel_spmd(nc, in_maps, core_ids=list(range(NCORES)),
                             trace=PROFILE)
    if PROFILE:
        LAST_PROFILE.append((name, r))
    return r.results


def make_p1_inputs(x, w_qkv, w_dw, temperature):
    ins = []
    wq64 = w_qkv.astype(np.float64)
    wd64 = w_dw.astype(np.float64).reshape(48, 9)
    for k in range(NCORES):
        b, g = divmod(k, 4)
        rows = ([4 * g + u for u in range(4)]
                + [16 + 4 * g + u for u in range(4)]
                + [32 + 4 * g + u for u in range(4)])
        # wt[(band,ci), t, o*8+band] = w_qkv[row_o, ci] * w_dw[row_o, t]
        wt = np.zeros((8, 16, 9, 12, 8), np.float64)
        for o in range(12):
            prod = np.einsum('c,t->tc', wq64[rows[o]], wd64[rows[o]])
            for band in range(8):
                wt[band, :, :, o, band] = prod.T
        wt = np.ascontiguousarray(
            wt.reshape(128, 9 * 96).astype(np.float32))
        wh = wt.astype(np.float16)
        wl = ((wt - wh.astype(np.float32)) * S_SPLIT).astype(np.float16)
        xpad = np.zeros((16, 258, 258), np.float32)
        xpad[:, 1:257, 1:257] = x[b]
        xh = xpad.astype(np.float16)
        xl = (xpad - xh.astype(np.float32)).astype(np.float16)
        xs = (xh.astype(np.float32) / S_SPLIT).astype(np.float16)
        ins.append({
            "xh": xh, "xl": xl, "xs": xs, "wh": wh, "wl": wl,
            "tempu": np.full((1, 1), temperature[g, 0, 0], np.float32),
        })
    return ins


def _host_eigh(cov_all):
    """cov_all: (32,256,256) f32 -> top-100 eigvecs via jax CPU f64 eigh."""
    import jax
    jax.config.update("jax_enable_x64", True)
    import jax.numpy as jnp
    cpu = jax.devices("cpu")[0]
    with jax.default_device(cpu):
        _, vecs = jnp.linalg.eigh(
            jax.device_put(jnp.asarray(cov_all.astype(np.float64)), cpu))
        U = np.asarray(vecs)[:, :, ::-1][:, :, :TOP_P]
    return U


def kernel(x, w_qkv, w_dw, temperature, w_fr, w_po):
    import ml_dtypes
    bf16 = ml_dtypes.bfloat16
    x = np.ascontiguousarray(np.asarray(x, dtype=np.float32))
    w_qkv = np.asarray(w_qkv, dtype=np.float32)
    w_dw = np.asarray(w_dw, dtype=np.float32)
    temperature = np.asarray(temperature, dtype=np.float32)
    w_fr = np.asarray(w_fr, dtype=np.float32)
    w_po = np.asarray(w_po, dtype=np.float32)

    # ---- L1
    nc1 = _get("p1")
    res1 = _run("p1", nc1, make_p1_inputs(x, w_qkv, w_dw, temperature))

    # ---- host: unpermute cov to natural patch order, eigh, repermute U
    cov_dev = np.zeros((B, C, 256, 256), np.float32)
    vpd = np.zeros((B, C, 128, 512), np.float32)
    for k in range(NCORES):
        b, g = divmod(k, 4)
        cov_dev[b, 4 * g:4 * g + 4] = res1[k]["cov"]
        vpd[b, 4 * g:4 * g + 4] = res1[k]["vpd"]
    cov_nat = cov_dev[:, :, P_INV][:, :, :, P_INV]
    DEBUG["cov_all"] = cov_nat
    U_nat = _host_eigh(cov_nat.reshape(-1, 256, 256))   # (32, 256, 100)
    U_dev = U_nat[:, P_NAT, :].reshape(B, C, 256, TOP_P)

    # ---- L2 (+ folded projection partials)
    nc2 = _get("p2")
    wfrT = np.ascontiguousarray(w_fr.T)
    in2 = []
    for k in range(NCORES):
        b, g = divmod(k, 4)
        ut = np.ascontiguousarray(
            U_dev[b, 4 * g:4 * g + 4].transpose(2, 0, 1).reshape(100, 1024)
        ).astype(np.float32)
        wpoD = np.zeros((32, 128), np.float32)
        for s in range(8):
            for u in range(4):
                wpoD[s * 4 + u, s * 16:s * 16 + 16] = w_po[:, 4 * g + u]
        in2.append({
            "ut": ut,
            "vpd": np.ascontiguousarray(vpd[b, 4 * g:4 * g + 4]),
            "wfrT": wfrT,
            "wpoD": wpoD.astype(bf16),
        })
    res2 = _run("p2", nc2, in2)

    # ---- host: sum partials, unpermute np, second patchify (P2)
    out = np.zeros((B, C, 256, 256), np.float32)
    for b in range(B):
        acc = np.zeros((128, 8192), np.float32)
        for g in range(4):
            acc += res2[4 * b + g]["po"].astype(np.float32)
        # partitions (sp8, o16); free (pl16, mc2, d256);
        # n_dev = mc*128 + sp*16 + pl
        m_dev = acc.reshape(8, 16, 16, 2, 256).transpose(1, 3, 0, 2, 4)
        m_dev = m_dev.reshape(16, 256, 256)
        m_nat = np.zeros_like(m_dev)
        m_nat[:, P_NAT, :] = m_dev
        DEBUG.setdefault("mperm", {})[b] = m_nat
        # P2: patchify the (np, d) matrix per channel
        out[b] = m_nat.reshape(16, 16, 16, 16, 16).transpose(
            0, 1, 3, 2, 4).reshape(16, 256, 256)
    DEBUG["mperm"] = out.copy()
    return out


# revision 15
# speedup vs baseline: 1.2587x; 1.2587x over previous
"""Trainium2 Bass kernel for nn_AttentionSpatial (manifold attention), v4.

Pipeline (per the reference):
  qkv = 1x1 conv -> 3x3 depthwise conv -> patchify -> per-(b,head,c) unit:
  normalize q,k -> attn = softmax(q k^T * temp) -> cov -> eigh (top-100)
  -> A = U (w_fr^T w_fr) U^T -> out = A v -> re-patchify -> 1x1 conv out.

Sharding: 32 units = (batch 2) x (channel 16); core k=(b,g) owns channels
4g..4g+4 of batch b.  Eigh runs on host (jax CPU f64) because the final
output depends on LAPACK eigenvector signs; everything else on device.

v4 changes vs v2 baseline:
  - conv in 3-pass f16 split precision (x=xh+xl, w=wh+wl; passes
    wh*xh + wh*xl + (wl*16)*(xh/16)); ~5e-7 rel error (vs f32 1e-7),
    3 cyc/row instead of f32's 4, and each pass streams at 1 cyc/row.
  - patchify moved fully on-chip: two PE-transpose stages image->np-major
    (device patch order n_dev = pb*128 + band*16 + pc; host unpermutes).
    Kills the DRAM round-trip + 64B-line gather DMAs.
  - v ships as np-major vpd directly (no host patchify).
  - p3 folded into p2: each core emits the 16-channel partial projection
    partial[o] = sum_u w_po[o,c_u] m_u (bf16); host sums 4 partials.
  - attn/cov matmuls stay f32 (eigh amplifies cov error ~1000x; f32r's
    1.5e-4 matmul error gives 0.2 final rel err - measured).
"""

import numpy as np

PATCH = 16
HEADS = 4
TOP_P = 100
B, C = 2, 16
NCORES = 8
S_SPLIT = 16.0

_built = {}
PROFILE = False
LAST_PROFILE = []
DEBUG = {}

# device patch order: n_dev = pb*128 + band*16 + pc  <->  nat = (2*band+pb)*16+pc
_i = np.arange(256)
P_NAT = ((2 * ((_i >> 4) & 7) + (_i >> 7)) * 16 + (_i & 15)).astype(np.int64)
P_INV = np.zeros(256, np.int64)
P_INV[P_NAT] = _i


def _new_nc():
    from concourse import bacc
    return bacc.Bacc("TRN2", target_bir_lowering=False, debug=False)


# --------------------------------------------------------------------------
# L1: 3-pass f16 conv + on-chip patchify + attention + cov
# --------------------------------------------------------------------------

def _build_p1():
    import concourse.bass as bass
    import concourse.tile as tile
    from concourse import mybir
    from concourse.masks import make_identity

    f32 = mybir.dt.float32
    f16 = mybir.dt.float16
    AF = mybir.ActivationFunctionType
    OP = mybir.AluOpType
    nc = _new_nc()

    xh_d = nc.dram_tensor("xh", (16, 258, 258), f16, kind="ExternalInput")
    xl_d = nc.dram_tensor("xl", (16, 258, 258), f16, kind="ExternalInput")
    xs_d = nc.dram_tensor("xs", (16, 258, 258), f16, kind="ExternalInput")
    wh_d = nc.dram_tensor("wh", (128, 9 * 96), f16, kind="ExternalInput")
    wl_d = nc.dram_tensor("wl", (128, 9 * 96), f16, kind="ExternalInput")
    tmp_d = nc.dram_tensor("tempu", (1, 1), f32, kind="ExternalInput")
    vpd_d = nc.dram_tensor("vpd", (4, 128, 512), f32, kind="ExternalOutput")
    cov_d = nc.dram_tensor("cov", (4, 256, 256), f32, kind="ExternalOutput")

    with tile.TileContext(nc) as tc:
        with (
            tc.tile_pool(name="big", bufs=1) as big,
            tc.tile_pool(name="pdp", bufs=1) as pdp,
            tc.tile_pool(name="unit", bufs=4) as up,
            tc.tile_pool(name="small", bufs=3) as sp,
            tc.tile_pool(name="psC", bufs=2, space="PSUM") as psC,
            tc.tile_pool(name="psTT", bufs=2, space="PSUM") as psTT,
            tc.tile_pool(name="psT", bufs=2, space="PSUM") as psT,
            tc.tile_pool(name="psA", bufs=2, space="PSUM") as psA,
        ):
            # ---- x in four chained slabs per tensor (xh/xl/xs on separate
            # queues); marker copies serialize each chain so slab 0 lands
            # at full bandwidth.
            SLABS = ((0, 8), (6, 16), (14, 24), (22, 34))
            xqs = {}
            for name, dram, eng in (("xh", xh_d, nc.sync),
                                    ("xl", xl_d, nc.scalar),
                                    ("xs", xs_d, nc.gpsimd)):
                tiles = []
                for q, (r0, r1) in enumerate(SLABS):
                    t_xq = big.tile([128, r1 - r0, 258], f16,
                                    tag=f"{name}{q}")
                    tiles.append(t_xq)
                for q, (r0, r1) in enumerate(SLABS):
                    if q > 0:
                        nc.vector.tensor_copy(tiles[q][0:1, 0, 0:1],
                                              tiles[q - 1][0:1, 0, 0:1])
                    eng.dma_start(
                        tiles[q],
                        bass.AP(tensor=dram.ap().tensor, offset=r0 * 258,
                                ap=[[32 * 258, 8], [258 * 258, 16],
                                    [1, (r1 - r0) * 258]]),
                    )
                xqs[name] = tiles

            wh = big.tile([128, 9, 96], f16, tag="wh")
            nc.sync.dma_start(wh.rearrange("p a b -> p (a b)"), wh_d.ap())
            wl = big.tile([128, 9, 96], f16, tag="wl")
            nc.scalar.dma_start(wl.rearrange("p a b -> p (a b)"), wl_d.ap())

            ident = big.tile([128, 128], f32, tag="ident")
            make_identity(nc, ident)
            tempb = big.tile([128, 1], f32, tag="tempb")
            nc.sync.dma_start(
                tempb,
                bass.AP(tensor=tmp_d.ap().tensor, offset=0,
                        ap=[[0, 128], [1, 1]]),
            )

            # ---- HAM warm-up: dense bf16 matmuls spanning the head so the
            # PE clock is ramped when the conv starts
            identb = big.tile([128, 128], mybir.dt.bfloat16, tag="identb")
            nc.vector.tensor_copy(identb, ident)
            junk = big.tile([128, 512], mybir.dt.bfloat16, tag="junk")
            nc.vector.memset(junk, 1.0)
            prm = psT.tile([128, 2, 256], f32, tag="tps")
            for i in range(32):
                nc.tensor.matmul(prm.rearrange("p a b -> p (a b)"), identb,
                                 junk, start=(i == 0), stop=(i == 31))

            # ---- conv: 32 chunks (one image row across 8 bands), 27 f16
            # matmuls each (9 taps x 3 split passes)
            # q2all[(o,band), pb, pc, dc0, dr, dcH]: T1 reads the
            # (dr,dcH) 128-block contiguously; the conv copy absorbs the
            # column permute
            q2all = big.tile([96, 2, 16, 2, 16, 8], f32, tag="q2all")
            taps = [(dy, dx) for dy in (-1, 0, 1) for dx in (-1, 0, 1)]

            def conv_pair(r):
                # two image rows per matmul: strided 2-row f16 rhs streams
                # at full rate (measured), halving instruction count
                acc = psC.tile([96, 512], f32, tag="conv")
                q = 0 if r < 6 else (1 if r < 14 else (2 if r < 22 else 3))
                r0 = SLABS[q][0]
                k = 0
                for t, (dy, dx) in enumerate(taps):
                    row = r - r0 + 1 + dy
                    cs = slice(dx + 1, dx + 257)
                    for wt_, xn in ((wh, "xh"), (wh, "xl"), (wl, "xs")):
                        nc.tensor.matmul(acc, wt_[:, t, :],
                                         xqs[xn][q][:, row:row + 2, cs],
                                         start=(k == 0), stop=(k == 26))
                        k += 1
                for rr in (0, 1):
                    rc = r + rr
                    dst = q2all[:, rc // 16, :, :, rc % 16, :]
                    src_v = acc[:, 256 * rr:256 * (rr + 1)].rearrange(
                        "p (a b c) -> p a c b", a=16, b=8)
                    if rr == 0:
                        nc.vector.tensor_copy(dst, src_v)
                    else:
                        nc.scalar.copy(dst, src_v)

            # ---- T1: [(o,band), (dr,dcH)-slice] -> [(dr,dcH), (o,band)]
            # per (pb, pc, dc0); wave pb=0 issued mid-conv order-wise
            mid = big.tile([128, 2, 12, 2, 8, 16], f32, tag="mid")
            # gpsimd cannot access PSUM; alternate vector/scalar
            CPY = [nc.vector.tensor_copy, nc.scalar.copy]

            def t1_wave(pb):
                for i, (pc, dc0) in enumerate(
                        (pc, dc0) for pc in range(16) for dc0 in range(2)):
                    tps = psTT.tile([128, 128], f32, tag="tt")
                    nc.tensor.transpose(
                        tps[:, 0:96],
                        q2all[:, pb, pc, dc0].rearrange("p a b -> p (a b)"),
                        ident[0:96, 0:96])
                    CPY[i % 2](
                        mid[:, pb, :, dc0, :, pc],
                        tps[:, 0:96].rearrange("p (o bd) -> p o bd", o=12))

            # ---- T2: [(dr,dcH), (band,pc)] -> np-major pd per channel
            pd = {}
            for o in range(12):
                t_pd = pdp.tile([128, 2, 16, 8, 2], f32, tag=f"pd{o}")
                pd[o] = t_pd

            def t2_wave(pb, o_list):
                for i, (o, dc0) in enumerate(
                        (o, dc0) for o in o_list for dc0 in range(2)):
                    tps = psTT.tile([128, 128], f32, tag="tt")
                    nc.tensor.transpose(
                        tps, mid[:, pb, o, dc0].rearrange("p a b -> p (a b)"),
                        ident)
                    CPY[i % 2](
                        pd[o][:, pb, :, :, dc0],
                        tps.rearrange("p (a b) -> p a b", a=16))

            # interleave waves with the conv halves: keeps the PE queue
            # dense (pstate stays high) and lets copies drain in parallel
            for r in range(0, 16, 2):
                conv_pair(r)
            t1_wave(0)
            for r in range(16, 24, 2):
                conv_pair(r)
            t2_wave(0, range(12))
            for r in range(24, 32, 2):
                conv_pair(r)
            t1_wave(1)

            def pdh(o, c2):
                return pd[o][:, c2].rearrange("p b c d -> p (b c d)")

            # ---- normalize q,k rows (np-major; norm over free=d)
            def normalize(o):
                scr = sp.tile([128, 256], f32, tag="scr")
                nrm2 = sp.tile([128, 2], f32, tag="nrm2")
                for c2 in range(2):
                    nc.scalar.activation(scr, pdh(o, c2), AF.Square,
                                         accum_out=nrm2[:, c2:c2 + 1])
                nc.vector.tensor_scalar_max(nrm2, nrm2, 1e-24)
                srt = sp.tile([128, 2], f32, tag="srt")
                nc.scalar.sqrt(srt, nrm2)
                rin = sp.tile([128, 2], f32, tag="rin")
                nc.vector.reciprocal(rin, srt)
                # one newton step on rsqrt fixes table-sqrt error:
                # r1 = r0*(1.5 - 0.5*n2*r0^2)
                nwt = sp.tile([128, 2], f32, tag="nwt")
                nc.vector.tensor_mul(nwt, rin, rin)
                nc.vector.tensor_mul(nwt, nwt, nrm2)
                nc.vector.tensor_scalar(nwt, nwt, -0.5, 1.5,
                                        op0=OP.mult, op1=OP.add)
                nc.vector.tensor_mul(rin, rin, nwt)
                for c2 in range(2):
                    nc.vector.tensor_scalar_mul(
                        pdh(o, c2), pdh(o, c2), rin[:, c2:c2 + 1])

            # ---- per unit: normalize + q/k transposes (T3) interleaved
            # T2 wave 1 per unit pair, normalize as soon as each channel
            # lands; v channels last, then their vpd DMAs
            for u in range(4):
                t2_wave(1, (u, 4 + u))
                normalize(u)
                normalize(4 + u)
            t2_wave(1, range(8, 12))
            for o in range(8, 12):
                eng = (nc.sync, nc.scalar, nc.gpsimd, nc.sync)[o - 8]
                eng.dma_start(
                    vpd_d.ap()[o - 8],
                    pd[o].rearrange("p a b c d -> p (a b c d)"))

            qTs, kTs, att_es, rssums, xcs, xcTs = {}, {}, {}, {}, {}, {}
            for u in range(4):
                qT = up.tile([128, 2, 256], f32, tag="qT")
                qTs[u] = qT
                kT = up.tile([128, 2, 256], f32, tag="kT")
                kTs[u] = kT
                for ti, (src_o, dst_t) in enumerate(((u, qT), (4 + u, kT))):
                    tps = psT.tile([128, 2, 256], f32, tag="tps")
                    for nh in range(2):
                        for dh in range(2):
                            nc.tensor.transpose(
                                tps[:, dh, 128 * nh:128 * (nh + 1)],
                                pd[src_o][:, nh, 8 * dh:8 * (dh + 1)]
                                .rearrange("p a b c -> p (a b c)"),
                                ident)
                    if ti == 0:
                        nc.vector.tensor_copy(dst_t, tps)
                    else:
                        nc.scalar.copy(dst_t, tps)

            for u in range(4):
                att_e = up.tile([128, 2, 256], f32, tag="att_e")
                att_es[u] = att_e
                rssum = sp.tile([128, 2], f32, tag=f"rssum{u}")
                rssums[u] = rssum
                for nh in range(2):
                    att = psA.tile([128, 256], f32, tag="att")
                    for dh in range(2):
                        nc.tensor.matmul(
                            att, qTs[u][:, dh, 128 * nh:128 * (nh + 1)],
                            kTs[u][:, dh, :], start=(dh == 0),
                            stop=(dh == 1))
                    nc.scalar.activation(att_e[:, nh, :], att, AF.Exp,
                                         scale=tempb[:, 0:1],
                                         accum_out=rssum[:, nh:nh + 1])

            for u in range(4):
                rinv = sp.tile([128, 2], f32, tag=f"rinv{u}")
                nc.vector.reciprocal(rinv, rssums[u])
                xc = up.tile([128, 2, 256], f32, tag="xc")
                xcs[u] = xc
                for nh in range(2):
                    nc.vector.tensor_scalar(
                        xc[:, nh, :], att_es[u][:, nh, :],
                        rinv[:, nh:nh + 1], 1.0 / 256.0,
                        op0=OP.mult, op1=OP.subtract)

            for u in range(4):
                xcT = up.tile([128, 2, 256], f32, tag="xcT")
                xcTs[u] = xcT
                tps2 = psT.tile([128, 2, 256], f32, tag="tps")
                for nh in range(2):
                    for mh in range(2):
                        nc.tensor.transpose(
                            tps2[:, mh, 128 * nh:128 * (nh + 1)],
                            xcs[u][:, nh, 128 * mh:128 * (mh + 1)], ident)
                if u % 2 == 0:
                    nc.vector.tensor_copy(xcT, tps2)
                else:
                    nc.scalar.copy(xcT, tps2)

            for u in range(4):
                xcT = xcTs[u]
                cov_sb = up.tile([128, 2, 256], f32, tag="cov_sb")
                for nh in range(2):
                    cv = psA.tile([128, 256], f32, tag="att")
                    for mh in range(2):
                        nc.tensor.matmul(
                            cv, xcT[:, mh, 128 * nh:128 * (nh + 1)],
                            xcT[:, mh, :], start=(mh == 0), stop=(mh == 1))
                    if nh == 0:
                        nc.vector.tensor_copy(cov_sb[:, nh, :], cv)
                    else:
                        nc.scalar.copy(cov_sb[:, nh, :], cv)
                cov_view = cov_d.ap()[u].rearrange("(c p) m -> p c m",
                                                   p=128)
                for nh in range(2):
                    eng = nc.scalar if (2 * u + nh) % 2 == 0 else nc.sync
                    eng.dma_start(cov_view[:, nh, :], cov_sb[:, nh, :])

    nc.compile()
    return nc


# --------------------------------------------------------------------------
# L2: yT = w_fr U^T, A = Y Y^T, M = A v, partial = sum_u wpo[:,c_u] M_u
# --------------------------------------------------------------------------

def _build_p2():
    import concourse.tile as tile
    from concourse import mybir

    f32 = mybir.dt.float32
    f32r = mybir.dt.float32r
    bf16 = mybir.dt.bfloat16
    nc = _new_nc()

    ut_d = nc.dram_tensor("ut", (100, 1024), f32r, kind="ExternalInput")
    vpd_d = nc.dram_tensor("vpd", (4, 128, 512), f32r, kind="ExternalInput")
    wfrT_d = nc.dram_tensor("wfrT", (100, 100), f32r, kind="ExternalInput")
    wpoD_d = nc.dram_tensor("wpoD", (32, 128), bf16, kind="ExternalInput")
    po_d = nc.dram_tensor("po", (128, 8192), bf16, kind="ExternalOutput")

    with tile.TileContext(nc) as tc:
        with (
            tc.tile_pool(name="sb", bufs=1) as sb,
            tc.tile_pool(name="unit", bufs=4) as up,
            tc.tile_pool(name="ps", bufs=2, space="PSUM") as ps,
            tc.tile_pool(name="psP", bufs=2, space="PSUM") as psP,
        ):
            wfrT = sb.tile([100, 100], f32r, tag="wfrT")
            nc.gpsimd.dma_start(wfrT, wfrT_d.ap())
            wpoD = sb.tile([32, 128], bf16, tag="wpoD")
            nc.gpsimd.dma_start(wpoD, wpoD_d.ap())
            ut_all = sb.tile([100, 4, 256], f32r, tag="ut")
            nc.sync.dma_start(
                ut_all.rearrange("p a b -> p (a b)"), ut_d.ap())
            v_all = sb.tile([128, 4, 2, 256], f32r, tag="v")
            nc.scalar.dma_start(
                v_all, vpd_d.ap().rearrange("a p b -> p a b"))

            mpack = sb.tile([32, 8192], bf16, tag="mpack")
            DQ = [nc.sync, nc.scalar, nc.gpsimd]
            # ramp the PE clock while inputs stream in
            junk = sb.tile([128, 512], bf16, tag="junk")
            nc.vector.memset(junk, 1.0)
            jw = sb.tile([128, 128], bf16, tag="jw")
            nc.vector.memset(jw, 0.5)
            pw = psP.tile([128, 512], f32, tag="pp")
            for i in range(14):
                nc.tensor.matmul(pw, jw, junk, start=(i == 0),
                                 stop=(i == 13))
            for u in range(4):
                yTp = ps.tile([100, 256], f32, tag="yT")
                nc.tensor.matmul(yTp, wfrT, ut_all[:, u, :],
                                 start=True, stop=True)
                yT = up.tile([100, 256], f32r, tag="yTs")
                nc.vector.tensor_copy(yT, yTp)

                a_sb = up.tile([128, 2, 256], f32r, tag="a_sb")
                for nh in range(2):
                    ap_ = ps.tile([128, 256], f32, tag="aps")
                    nc.tensor.matmul(ap_, yT[:, 128 * nh:128 * (nh + 1)],
                                     yT, start=True, stop=True)
                    if nh == 0:
                        nc.scalar.copy(a_sb[:, nh, :], ap_)
                    else:
                        nc.vector.tensor_copy(a_sb[:, nh, :], ap_)

                m_sb = up.tile([128, 2, 256], bf16, tag="m_sb")
                for mc in range(2):
                    mp = ps.tile([128, 256], f32, tag="mps")
                    for kc in range(2):
                        nc.tensor.matmul(
                            mp, a_sb[:, kc, 128 * mc:128 * (mc + 1)],
                            v_all[:, u, kc, :], start=(kc == 0),
                            stop=(kc == 1))
                    if mc == 0:
                        nc.scalar.copy(m_sb[:, mc, :], mp)
                    else:
                        nc.vector.tensor_copy(m_sb[:, mc, :], mp)
                # pack m_u rows into partitions (sp*4+u) for the
                # channel-mix contraction
                for s in range(8):
                    DQ[s % 3].dma_start(
                        mpack[4 * s + u:4 * s + u + 1, :],
                        m_sb[16 * s:16 * (s + 1)])

            # partial[o] = sum_u wpo[o,c_u] m_u  (block-diag over 8
            # partition groups)
            po_sb = sb.tile([128, 8192], bf16, tag="po_sb")
            for ch in range(16):
                pp = psP.tile([128, 512], f32, tag="pp")
                nc.tensor.matmul(pp, wpoD, mpack[:, 512 * ch:512 * (ch + 1)],
                                 start=True, stop=True)
                if ch % 2 == 0:
                    nc.vector.tensor_copy(
                        po_sb[:, 512 * ch:512 * (ch + 1)], pp)
                else:
                    nc.scalar.copy(po_sb[:, 512 * ch:512 * (ch + 1)], pp)
            for i in range(4):
                DQ[i % 3].dma_start(po_d.ap()[:, 2048 * i:2048 * (i + 1)],
                                    po_sb[:, 2048 * i:2048 * (i + 1)])

    nc.compile()
    return nc


# --------------------------------------------------------------------------
# host orchestration
# --------------------------------------------------------------------------

def _get(name):
    if name not in _built:
        _built[name] = {"p1": _build_p1, "p2": _build_p2}[name]()
    return _built[name]


def _run(name, nc, in_maps):
    from concourse.bass_utils import run_bass_kernel_spmd
    r = run_bass_kernel_spmd(nc, in_maps, core_ids=list(range(NCORES)),
                             trace=PROFILE)
    if PROFILE:
        LAST_PROFILE.append((name, r))
    return r.results


def make_p1_inputs(x, w_qkv, w_dw, temperature):
    ins = []
    wq64 = w_qkv.astype(np.float64)
    wd64 = w_dw.astype(np.float64).reshape(48, 9)
    for k in range(NCORES):
        b, g = divmod(k, 4)
        rows = ([4 * g + u for u in range(4)]
                + [16 + 4 * g + u for u in range(4)]
                + [32 + 4 * g + u for u in range(4)])
        # wt[(band,ci), t, o*8+band] = w_qkv[row_o, ci] * w_dw[row_o, t]
        wt = np.zeros((8, 16, 9, 12, 8), np.float64)
        for o in range(12):
            prod = np.einsum('c,t->tc', wq64[rows[o]], wd64[rows[o]])
            for band in range(8):
                wt[band, :, :, o, band] = prod.T
        wt = np.ascontiguousarray(
            wt.reshape(128, 9 * 96).astype(np.float32))
        wh = wt.astype(np.float16)
        wl = ((wt - wh.astype(np.float32)) * S_SPLIT).astype(np.float16)
        xpad = np.zeros((16, 258, 258), np.float32)
        xpad[:, 1:257, 1:257] = x[b]
        xh = xpad.astype(np.float16)
        xl = (xpad - xh.astype(np.float32)).astype(np.float16)
        xs = (xh.astype(np.float32) / S_SPLIT).astype(np.float16)
        ins.append({
            "xh": xh, "xl": xl, "xs": xs, "wh": wh, "wl": wl,
            "tempu": np.full((1, 1), temperature[g, 0, 0], np.float32),
        })
    return ins


def _host_eigh(cov_all):
    """cov_all: (32,256,256) f32 -> top-100 eigvecs via jax CPU f64 eigh."""
    import jax
    jax.config.update("jax_enable_x64", True)
    import jax.numpy as jnp
    cpu = jax.devices("cpu")[0]
    with jax.default_device(cpu):
        _, vecs = jnp.linalg.eigh(
            jax.device_put(jnp.asarray(cov_all.astype(np.float64)), cpu))
        U = np.asarray(vecs)[:, :, ::-1][:, :, :TOP_P]
    return U


def kernel(x, w_qkv, w_dw, temperature, w_fr, w_po):
    import ml_dtypes
    bf16 = ml_dtypes.bfloat16
    x = np.ascontiguousarray(np.asarray(x, dtype=np.float32))
    w_qkv = np.asarray(w_qkv, dtype=np.float32)
    w_dw = np.asarray(w_dw, dtype=np.float32)
    temperature = np.asarray(temperature, dtype=np.float32)
    w_fr = np.asarray(w_fr, dtype=np.float32)
    w_po = np.asarray(w_po, dtype=np.float32)

    # ---- L1
    nc1 = _get("p1")
    res1 = _run("p1", nc1, make_p1_inputs(x, w_qkv, w_dw, temperature))

    # ---- host: unpermute cov to natural patch order, eigh, repermute U
    cov_dev = np.zeros((B, C, 256, 256), np.float32)
    vpd = np.zeros((B, C, 128, 512), np.float32)
    for k in range(NCORES):
        b, g = divmod(k, 4)
        cov_dev[b, 4 * g:4 * g + 4] = res1[k]["cov"]
        vpd[b, 4 * g:4 * g + 4] = res1[k]["vpd"]
    cov_nat = cov_dev[:, :, P_INV][:, :, :, P_INV]
    DEBUG["cov_all"] = cov_nat
    U_nat = _host_eigh(cov_nat.reshape(-1, 256, 256))   # (32, 256, 100)
    U_dev = U_nat[:, P_NAT, :].reshape(B, C, 256, TOP_P)

    # ---- L2 (+ folded projection partials)
    nc2 = _get("p2")
    wfrT = np.ascontiguousarray(w_fr.T)
    in2 = []
    for k in range(NCORES):
        b, g = divmod(k, 4)
        ut = np.ascontiguousarray(
            U_dev[b, 4 * g:4 * g + 4].transpose(2, 0, 1).reshape(100, 1024)
        ).astype(np.float32)
        wpoD = np.zeros((32, 128), np.float32)
        for s in range(8):
            for u in range(4):
                wpoD[s * 4 + u, s * 16:s * 16 + 16] = w_po[:, 4 * g + u]
        in2.append({
            "ut": ut,
            "vpd": np.ascontiguousarray(vpd[b, 4 * g:4 * g + 4]),
            "wfrT": wfrT,
            "wpoD": wpoD.astype(bf16),
        })
    res2 = _run("p2", nc2, in2)

    # ---- host: sum partials, unpermute np, second patchify (P2)
    out = np.zeros((B, C, 256, 256), np.float32)
    for b in range(B):
        acc = np.zeros((128, 8192), np.float32)
        for g in range(4):
            acc += res2[4 * b + g]["po"].astype(np.float32)
        # partitions (sp8, o16); free (pl16, mc2, d256);
        # n_dev = mc*128 + sp*16 + pl
        m_dev = acc.reshape(8, 16, 16, 2, 256).transpose(1, 3, 0, 2, 4)
        m_dev = m_dev.reshape(16, 256, 256)
        m_nat = np.zeros_like(m_dev)
        m_nat[:, P_NAT, :] = m_dev
        DEBUG.setdefault("mperm", {})[b] = m_nat
        # P2: patchify the (np, d) matrix per channel
        out[b] = m_nat.reshape(16, 16, 16, 16, 16).transpose(
            0, 1, 3, 2, 4).reshape(16, 256, 256)
    DEBUG["mperm"] = out.copy()
    return out


# revision 17
# speedup vs baseline: 1.3547x; 1.0762x over previous
"""Trainium2 Bass kernel for nn_AttentionSpatial (manifold attention), v4.

Pipeline (per the reference):
  qkv = 1x1 conv -> 3x3 depthwise conv -> patchify -> per-(b,head,c) unit:
  normalize q,k -> attn = softmax(q k^T * temp) -> cov -> eigh (top-100)
  -> A = U (w_fr^T w_fr) U^T -> out = A v -> re-patchify -> 1x1 conv out.

Sharding: 32 units = (batch 2) x (channel 16); core k=(b,g) owns channels
4g..4g+4 of batch b.  Eigh runs on host (jax CPU f64) because the final
output depends on LAPACK eigenvector signs; everything else on device.

v4 changes vs v2 baseline:
  - conv in 3-pass f16 split precision (x=xh+xl, w=wh+wl; passes
    wh*xh + wh*xl + (wl*16)*(xh/16)); ~5e-7 rel error (vs f32 1e-7),
    3 cyc/row instead of f32's 4, and each pass streams at 1 cyc/row.
  - patchify moved fully on-chip: two PE-transpose stages image->np-major
    (device patch order n_dev = pb*128 + band*16 + pc; host unpermutes).
    Kills the DRAM round-trip + 64B-line gather DMAs.
  - v ships as np-major vpd directly (no host patchify).
  - p3 folded into p2: each core emits the 16-channel partial projection
    partial[o] = sum_u w_po[o,c_u] m_u (bf16); host sums 4 partials.
  - attn/cov matmuls stay f32 (eigh amplifies cov error ~1000x; f32r's
    1.5e-4 matmul error gives 0.2 final rel err - measured).
"""

import numpy as np

PATCH = 16
HEADS = 4
TOP_P = 100
B, C = 2, 16
NCORES = 8
S_SPLIT = 16.0

_built = {}
PROFILE = False
LAST_PROFILE = []
DEBUG = {}

# device patch order: n_dev = pb*128 + band*16 + pc  <->  nat = (2*band+pb)*16+pc
_i = np.arange(256)
P_NAT = ((2 * ((_i >> 4) & 7) + (_i >> 7)) * 16 + (_i & 15)).astype(np.int64)
P_INV = np.zeros(256, np.int64)
P_INV[P_NAT] = _i


def _new_nc():
    from concourse import bacc
    return bacc.Bacc("TRN2", target_bir_lowering=False, debug=False)


# --------------------------------------------------------------------------
# L1: 3-pass f16 conv + on-chip patchify + attention + cov
# --------------------------------------------------------------------------

def _build_p1():
    import concourse.bass as bass
    import concourse.tile as tile
    from concourse import mybir
    from concourse.masks import make_identity

    f32 = mybir.dt.float32
    f16 = mybir.dt.float16
    AF = mybir.ActivationFunctionType
    OP = mybir.AluOpType
    nc = _new_nc()

    xh_d = nc.dram_tensor("xh", (16, 258, 258), f16, kind="ExternalInput")
    xl_d = nc.dram_tensor("xl", (16, 258, 258), f16, kind="ExternalInput")
    xs_d = nc.dram_tensor("xs", (16, 258, 258), f16, kind="ExternalInput")
    wh_d = nc.dram_tensor("wh", (128, 9 * 96), f16, kind="ExternalInput")
    wl_d = nc.dram_tensor("wl", (128, 9 * 96), f16, kind="ExternalInput")
    tmp_d = nc.dram_tensor("tempu", (1, 1), f32, kind="ExternalInput")
    vpd_d = nc.dram_tensor("vpd", (4, 128, 512), f32, kind="ExternalOutput")
    cov_d = nc.dram_tensor("cov", (4, 256, 256), f32, kind="ExternalOutput")

    with tile.TileContext(nc) as tc:
        with (
            tc.tile_pool(name="big", bufs=1) as big,
            tc.tile_pool(name="pdp", bufs=1) as pdp,
            tc.tile_pool(name="unit", bufs=4) as up,
            tc.tile_pool(name="small", bufs=3) as sp,
            tc.tile_pool(name="psC", bufs=2, space="PSUM") as psC,
            tc.tile_pool(name="psTT", bufs=2, space="PSUM") as psTT,
            tc.tile_pool(name="psT", bufs=2, space="PSUM") as psT,
            tc.tile_pool(name="psA", bufs=2, space="PSUM") as psA,
        ):
            # ---- x in four chained slabs per tensor (xh/xl/xs on separate
            # queues); marker copies serialize each chain so slab 0 lands
            # at full bandwidth.
            wh = big.tile([128, 9, 96], f16, tag="wh")
            nc.sync.dma_start(wh.rearrange("p a b -> p (a b)"), wh_d.ap())
            wl = big.tile([128, 9, 96], f16, tag="wl")
            nc.scalar.dma_start(wl.rearrange("p a b -> p (a b)"), wl_d.ap())

            SLABS = ((0, 8), (6, 16), (14, 24), (22, 34))
            CHQ = {"xh": nc.sync, "xl": nc.scalar, "xs": nc.gpsimd}
            xqs = {n: [] for n in CHQ}
            for name in CHQ:
                for q, (r0, r1) in enumerate(SLABS):
                    t_xq = big.tile([128, r1 - r0, 258], f16,
                                    tag=f"{name}{q}")
                    xqs[name].append(t_xq)
            for q, (r0, r1) in enumerate(SLABS):
                for name, dram in (("xh", xh_d), ("xl", xl_d),
                                   ("xs", xs_d)):
                    if q > 0:
                        # chain markers all live on gpsimd so no busy
                        # queue cross-serializes the three chains
                        nc.gpsimd.tensor_copy(
                            xqs[name][q][0:1, 0, 0:1],
                            xqs[name][q - 1][0:1, 0, 0:1])
                    CHQ[name].dma_start(
                        xqs[name][q],
                        bass.AP(tensor=dram.ap().tensor, offset=r0 * 258,
                                ap=[[32 * 258, 8], [258 * 258, 16],
                                    [1, (r1 - r0) * 258]]),
                    )

            ident = big.tile([128, 128], f32, tag="ident")
            make_identity(nc, ident)
            tempb = big.tile([128, 1], f32, tag="tempb")
            nc.sync.dma_start(
                tempb,
                bass.AP(tensor=tmp_d.ap().tensor, offset=0,
                        ap=[[0, 128], [1, 1]]),
            )

            # ---- HAM warm-up: dense bf16 matmuls spanning the head so the
            # PE clock is ramped when the conv starts
            identb = big.tile([128, 128], mybir.dt.bfloat16, tag="identb")
            nc.vector.tensor_copy(identb, ident)
            junk = big.tile([128, 512], mybir.dt.bfloat16, tag="junk")
            nc.vector.memset(junk, 1.0)
            prm = psT.tile([128, 2, 256], f32, tag="tps")
            for i in range(22):
                nc.tensor.matmul(prm.rearrange("p a b -> p (a b)"), identb,
                                 junk, start=(i == 0), stop=(i == 21))

            # ---- conv: 32 chunks (one image row across 8 bands), 27 f16
            # matmuls each (9 taps x 3 split passes)
            # q2all[(o,band), pb, pc, dc0, dr, dcH]: T1 reads the
            # (dr,dcH) 128-block contiguously; the conv copy absorbs the
            # column permute
            q2all = big.tile([96, 2, 16, 2, 16, 8], f32, tag="q2all")
            taps = [(dy, dx) for dy in (-1, 0, 1) for dx in (-1, 0, 1)]

            def conv_pair(r):
                # two image rows per matmul: strided 2-row f16 rhs streams
                # at full rate (measured), halving instruction count
                acc = psC.tile([96, 512], f32, tag="conv")
                q = 0 if r < 6 else (1 if r < 14 else (2 if r < 22 else 3))
                r0 = SLABS[q][0]
                k = 0
                for t, (dy, dx) in enumerate(taps):
                    row = r - r0 + 1 + dy
                    cs = slice(dx + 1, dx + 257)
                    for wt_, xn in ((wh, "xh"), (wh, "xl"), (wl, "xs")):
                        nc.tensor.matmul(acc, wt_[:, t, :],
                                         xqs[xn][q][:, row:row + 2, cs],
                                         start=(k == 0), stop=(k == 26))
                        k += 1
                for rr in (0, 1):
                    rc = r + rr
                    dst = q2all[:, rc // 16, :, :, rc % 16, :]
                    src_v = acc[:, 256 * rr:256 * (rr + 1)].rearrange(
                        "p (a b c) -> p a c b", a=16, b=8)
                    nc.vector.tensor_copy(dst, src_v)

            # ---- T1: [(o,band), (dr,dcH)-slice] -> [(dr,dcH), (o,band)]
            # per (pb, pc, dc0); wave pb=0 issued mid-conv order-wise
            mid = big.tile([128, 2, 12, 2, 8, 16], f32, tag="mid")
            # gpsimd cannot access PSUM; alternate vector/scalar
            CPY = [nc.vector.tensor_copy, nc.scalar.copy]

            def t1_wave(pb):
                for i, (pc, dc0) in enumerate(
                        (pc, dc0) for pc in range(16) for dc0 in range(2)):
                    tps = psTT.tile([128, 128], f32, tag="tt")
                    nc.tensor.transpose(
                        tps[:, 0:96],
                        q2all[:, pb, pc, dc0].rearrange("p a b -> p (a b)"),
                        ident[0:96, 0:96])
                    CPY[i % 2](
                        mid[:, pb, :, dc0, :, pc],
                        tps[:, 0:96].rearrange("p (o bd) -> p o bd", o=12))

            # ---- T2: [(dr,dcH), (band,pc)] -> np-major pd per channel
            pd = {}
            for o in range(12):
                t_pd = pdp.tile([128, 2, 16, 8, 2], f32, tag=f"pd{o}")
                pd[o] = t_pd

            def t2_wave(pb, o_list):
                for i, (o, dc0) in enumerate(
                        (o, dc0) for o in o_list for dc0 in range(2)):
                    tps = psTT.tile([128, 128], f32, tag="tt")
                    nc.tensor.transpose(
                        tps, mid[:, pb, o, dc0].rearrange("p a b -> p (a b)"),
                        ident)
                    CPY[i % 2](
                        pd[o][:, pb, :, :, dc0],
                        tps.rearrange("p (a b) -> p a b", a=16))

            # interleave waves with the conv halves: keeps the PE queue
            # dense (pstate stays high) and lets copies drain in parallel
            for r in range(0, 16, 2):
                conv_pair(r)
            t1_wave(0)
            for r in range(16, 24, 2):
                conv_pair(r)
            t2_wave(0, range(12))
            for r in range(24, 32, 2):
                conv_pair(r)
            t1_wave(1)

            def pdh(o, c2):
                return pd[o][:, c2].rearrange("p b c d -> p (b c d)")

            # ---- normalize q,k rows (np-major; norm over free=d)
            def normalize(o):
                scr = sp.tile([128, 256], f32, tag="scr")
                nrm2 = sp.tile([128, 2], f32, tag="nrm2")
                for c2 in range(2):
                    nc.scalar.activation(scr, pdh(o, c2), AF.Square,
                                         accum_out=nrm2[:, c2:c2 + 1])
                nc.vector.tensor_scalar_max(nrm2, nrm2, 1e-24)
                srt = sp.tile([128, 2], f32, tag="srt")
                nc.scalar.sqrt(srt, nrm2)
                rin = sp.tile([128, 2], f32, tag="rin")
                nc.vector.reciprocal(rin, srt)
                # one newton step on rsqrt fixes table-sqrt error:
                # r1 = r0*(1.5 - 0.5*n2*r0^2)
                nwt = sp.tile([128, 2], f32, tag="nwt")
                nc.vector.tensor_mul(nwt, rin, rin)
                nc.vector.tensor_mul(nwt, nwt, nrm2)
                nc.vector.tensor_scalar(nwt, nwt, -0.5, 1.5,
                                        op0=OP.mult, op1=OP.add)
                nc.vector.tensor_mul(rin, rin, nwt)
                for c2 in range(2):
                    nc.vector.tensor_scalar_mul(
                        pdh(o, c2), pdh(o, c2), rin[:, c2:c2 + 1])

            # ---- per unit: normalize + q/k transposes (T3) interleaved
            # T2 wave 1 per unit pair, normalize as soon as each channel
            # lands; v channels last, then their vpd DMAs
            for u in range(4):
                t2_wave(1, (u, 4 + u))
                normalize(u)
                normalize(4 + u)
            t2_wave(1, range(8, 12))
            for o in range(8, 12):
                eng = (nc.sync, nc.scalar, nc.gpsimd, nc.sync)[o - 8]
                eng.dma_start(
                    vpd_d.ap()[o - 8],
                    pd[o].rearrange("p a b c d -> p (a b c d)"))

            qTs, kTs, att_es, rssums, xcs, xcTs = {}, {}, {}, {}, {}, {}
            for u in range(4):
                qT = up.tile([128, 2, 256], f32, tag="qT")
                qTs[u] = qT
                kT = up.tile([128, 2, 256], f32, tag="kT")
                kTs[u] = kT
                for ti, (src_o, dst_t) in enumerate(((u, qT), (4 + u, kT))):
                    tps = psT.tile([128, 2, 256], f32, tag="tps")
                    for nh in range(2):
                        for dh in range(2):
                            nc.tensor.transpose(
                                tps[:, dh, 128 * nh:128 * (nh + 1)],
                                pd[src_o][:, nh, 8 * dh:8 * (dh + 1)]
                                .rearrange("p a b c -> p (a b c)"),
                                ident)
                    if ti == 0:
                        nc.vector.tensor_copy(dst_t, tps)
                    else:
                        nc.scalar.copy(dst_t, tps)

            for u in range(4):
                att_e = up.tile([128, 2, 256], f32, tag="att_e")
                att_es[u] = att_e
                rssum = sp.tile([128, 2], f32, tag=f"rssum{u}")
                rssums[u] = rssum
                for nh in range(2):
                    att = psA.tile([128, 256], f32, tag="att")
                    for dh in range(2):
                        nc.tensor.matmul(
                            att, qTs[u][:, dh, 128 * nh:128 * (nh + 1)],
                            kTs[u][:, dh, :], start=(dh == 0),
                            stop=(dh == 1))
                    nc.scalar.activation(att_e[:, nh, :], att, AF.Exp,
                                         scale=tempb[:, 0:1],
                                         accum_out=rssum[:, nh:nh + 1])

            for u in range(4):
                rinv = sp.tile([128, 2], f32, tag=f"rinv{u}")
                nc.vector.reciprocal(rinv, rssums[u])
                xc = up.tile([128, 2, 256], f32, tag="xc")
                xcs[u] = xc
                for nh in range(2):
                    nc.vector.tensor_scalar(
                        xc[:, nh, :], att_es[u][:, nh, :],
                        rinv[:, nh:nh + 1], 1.0 / 256.0,
                        op0=OP.mult, op1=OP.subtract)

            for u in range(4):
                xcT = up.tile([128, 2, 256], f32, tag="xcT")
                xcTs[u] = xcT
                tps2 = psT.tile([128, 2, 256], f32, tag="tps")
                for nh in range(2):
                    for mh in range(2):
                        nc.tensor.transpose(
                            tps2[:, mh, 128 * nh:128 * (nh + 1)],
                            xcs[u][:, nh, 128 * mh:128 * (mh + 1)], ident)
                if u % 2 == 0:
                    nc.vector.tensor_copy(xcT, tps2)
                else:
                    nc.scalar.copy(xcT, tps2)

            for u in range(4):
                xcT = xcTs[u]
                cov_sb = up.tile([128, 2, 256], f32, tag="cov_sb")
                for nh in range(2):
                    cv = psA.tile([128, 256], f32, tag="att")
                    for mh in range(2):
                        nc.tensor.matmul(
                            cv, xcT[:, mh, 128 * nh:128 * (nh + 1)],
                            xcT[:, mh, :], start=(mh == 0), stop=(mh == 1))
                    if nh == 0:
                        nc.vector.tensor_copy(cov_sb[:, nh, :], cv)
                    else:
                        nc.scalar.copy(cov_sb[:, nh, :], cv)
                cov_view = cov_d.ap()[u].rearrange("(c p) m -> p c m",
                                                   p=128)
                for nh in range(2):
                    eng = nc.scalar if (2 * u + nh) % 2 == 0 else nc.sync
                    eng.dma_start(cov_view[:, nh, :], cov_sb[:, nh, :])

    nc.compile()
    return nc


# --------------------------------------------------------------------------
# L2: yT = w_fr U^T, A = Y Y^T, M = A v, partial = sum_u wpo[:,c_u] M_u
# --------------------------------------------------------------------------

def _build_p2():
    import concourse.tile as tile
    from concourse import mybir

    f32 = mybir.dt.float32
    f32r = mybir.dt.float32r
    bf16 = mybir.dt.bfloat16
    nc = _new_nc()

    ut_d = nc.dram_tensor("ut", (100, 1024), f32r, kind="ExternalInput")
    vpd_d = nc.dram_tensor("vpd", (4, 128, 512), f32r, kind="ExternalInput")
    wfrT_d = nc.dram_tensor("wfrT", (100, 100), f32r, kind="ExternalInput")
    wpoD_d = nc.dram_tensor("wpoD", (32, 128), bf16, kind="ExternalInput")
    po_d = nc.dram_tensor("po", (128, 8192), bf16, kind="ExternalOutput")

    with tile.TileContext(nc) as tc:
        with (
            tc.tile_pool(name="sb", bufs=1) as sb,
            tc.tile_pool(name="unit", bufs=4) as up,
            tc.tile_pool(name="ps", bufs=2, space="PSUM") as ps,
            tc.tile_pool(name="psP", bufs=2, space="PSUM") as psP,
        ):
            wfrT = sb.tile([100, 100], f32r, tag="wfrT")
            nc.gpsimd.dma_start(wfrT, wfrT_d.ap())
            wpoD = sb.tile([32, 128], bf16, tag="wpoD")
            nc.gpsimd.dma_start(wpoD, wpoD_d.ap())
            ut_all = sb.tile([100, 4, 256], f32r, tag="ut")
            nc.sync.dma_start(
                ut_all.rearrange("p a b -> p (a b)"), ut_d.ap())
            v_all = sb.tile([128, 4, 2, 256], f32r, tag="v")
            nc.scalar.dma_start(
                v_all, vpd_d.ap().rearrange("a p b -> p a b"))

            mpack = sb.tile([32, 8192], bf16, tag="mpack")
            DQ = [nc.sync, nc.scalar, nc.gpsimd]
            # ramp the PE clock while inputs stream in
            junk = sb.tile([128, 512], bf16, tag="junk")
            nc.vector.memset(junk, 1.0)
            jw = sb.tile([128, 128], bf16, tag="jw")
            nc.vector.memset(jw, 0.5)
            pw = psP.tile([128, 512], f32, tag="pp")
            for i in range(14):
                nc.tensor.matmul(pw, jw, junk, start=(i == 0),
                                 stop=(i == 13))
            yTs, a_sbs, m_sbs = {}, {}, {}
            for u in range(4):
                yTp = ps.tile([100, 256], f32, tag="yT")
                nc.tensor.matmul(yTp, wfrT, ut_all[:, u, :],
                                 start=True, stop=True)
                yT = up.tile([100, 256], f32r, tag="yTs")
                yTs[u] = yT
                if u % 2 == 0:
                    nc.vector.tensor_copy(yT, yTp)
                else:
                    nc.scalar.copy(yT, yTp)
            for u in range(4):
                a_sb = up.tile([128, 2, 256], f32r, tag="a_sb")
                a_sbs[u] = a_sb
                for nh in range(2):
                    ap_ = ps.tile([128, 256], f32, tag="aps")
                    nc.tensor.matmul(
                        ap_, yTs[u][:, 128 * nh:128 * (nh + 1)],
                        yTs[u], start=True, stop=True)
                    if nh == 0:
                        nc.scalar.copy(a_sb[:, nh, :], ap_)
                    else:
                        nc.vector.tensor_copy(a_sb[:, nh, :], ap_)
            for u in range(4):
                m_sb = up.tile([128, 2, 256], bf16, tag="m_sb")
                m_sbs[u] = m_sb
                for mc in range(2):
                    mp = ps.tile([128, 256], f32, tag="mps")
                    for kc in range(2):
                        nc.tensor.matmul(
                            mp, a_sbs[u][:, kc, 128 * mc:128 * (mc + 1)],
                            v_all[:, u, kc, :], start=(kc == 0),
                            stop=(kc == 1))
                    if mc == 0:
                        nc.scalar.copy(m_sb[:, mc, :], mp)
                    else:
                        nc.vector.tensor_copy(m_sb[:, mc, :], mp)
                # pack m_u rows into partitions (sp*4+u) for the
                # channel-mix contraction
                for s in range(8):
                    DQ[s % 3].dma_start(
                        mpack[4 * s + u:4 * s + u + 1, :],
                        m_sbs[u][16 * s:16 * (s + 1)])

            # partial[o] = sum_u wpo[o,c_u] m_u  (block-diag over 8
            # partition groups)
            po_sb = sb.tile([128, 8192], bf16, tag="po_sb")
            for ch in range(16):
                pp = psP.tile([128, 512], f32, tag="pp")
                nc.tensor.matmul(pp, wpoD, mpack[:, 512 * ch:512 * (ch + 1)],
                                 start=True, stop=True)
                if ch % 2 == 0:
                    nc.vector.tensor_copy(
                        po_sb[:, 512 * ch:512 * (ch + 1)], pp)
                else:
                    nc.scalar.copy(po_sb[:, 512 * ch:512 * (ch + 1)], pp)
                if ch % 4 == 3:
                    i = ch // 4
                    DQ[i % 3].dma_start(
                        po_d.ap()[:, 2048 * i:2048 * (i + 1)],
                        po_sb[:, 2048 * i:2048 * (i + 1)])

    nc.compile()
    return nc


# --------------------------------------------------------------------------
# host orchestration
# --------------------------------------------------------------------------

def _get(name):
    if name not in _built:
        _built[name] = {"p1": _build_p1, "p2": _build_p2}[name]()
    return _built[name]


def _run(name, nc, in_maps):
    from concourse.bass_utils import run_bass_kernel_spmd
    r = run_bass_kernel_spmd(nc, in_maps, core_ids=list(range(NCORES)),
                             trace=PROFILE)
    if PROFILE:
        LAST_PROFILE.append((name, r))
    return r.results


def make_p1_inputs(x, w_qkv, w_dw, temperature):
    ins = []
    wq64 = w_qkv.astype(np.float64)
    wd64 = w_dw.astype(np.float64).reshape(48, 9)
    for k in range(NCORES):
        b, g = divmod(k, 4)
        rows = ([4 * g + u for u in range(4)]
                + [16 + 4 * g + u for u in range(4)]
                + [32 + 4 * g + u for u in range(4)])
        # wt[(band,ci), t, o*8+band] = w_qkv[row_o, ci] * w_dw[row_o, t]
        wt = np.zeros((8, 16, 9, 12, 8), np.float64)
        for o in range(12):
            prod = np.einsum('c,t->tc', wq64[rows[o]], wd64[rows[o]])
            for band in range(8):
                wt[band, :, :, o, band] = prod.T
        wt = np.ascontiguousarray(
            wt.reshape(128, 9 * 96).astype(np.float32))
        wh = wt.astype(np.float16)
        wl = ((wt - wh.astype(np.float32)) * S_SPLIT).astype(np.float16)
        xpad = np.zeros((16, 258, 258), np.float32)
        xpad[:, 1:257, 1:257] = x[b]
        xh = xpad.astype(np.float16)
        xl = (xpad - xh.astype(np.float32)).astype(np.float16)
        xs = (xh.astype(np.float32) / S_SPLIT).astype(np.float16)
        ins.append({
            "xh": xh, "xl": xl, "xs": xs, "wh": wh, "wl": wl,
            "tempu": np.full((1, 1), temperature[g, 0, 0], np.float32),
        })
    return ins


def _host_eigh(cov_all):
    """cov_all: (32,256,256) f32 -> top-100 eigvecs via jax CPU f64 eigh."""
    import jax
    jax.config.update("jax_enable_x64", True)
    import jax.numpy as jnp
    cpu = jax.devices("cpu")[0]
    with jax.default_device(cpu):
        _, vecs = jnp.linalg.eigh(
            jax.device_put(jnp.asarray(cov_all.astype(np.float64)), cpu))
        U = np.asarray(vecs)[:, :, ::-1][:, :, :TOP_P]
    return U


def kernel(x, w_qkv, w_dw, temperature, w_fr, w_po):
    import ml_dtypes
    bf16 = ml_dtypes.bfloat16
    x = np.ascontiguousarray(np.asarray(x, dtype=np.float32))
    w_qkv = np.asarray(w_qkv, dtype=np.float32)
    w_dw = np.asarray(w_dw, dtype=np.float32)
    temperature = np.asarray(temperature, dtype=np.float32)
    w_fr = np.asarray(w_fr, dtype=np.float32)
    w_po = np.asarray(w_po, dtype=np.float32)

    # ---- L1
    nc1 = _get("p1")
    res1 = _run("p1", nc1, make_p1_inputs(x, w_qkv, w_dw, temperature))

    # ---- host: unpermute cov to natural patch order, eigh, repermute U
    cov_dev = np.zeros((B, C, 256, 256), np.float32)
    vpd = np.zeros((B, C, 128, 512), np.float32)
    for k in range(NCORES):
        b, g = divmod(k, 4)
        cov_dev[b, 4 * g:4 * g + 4] = res1[k]["cov"]
        vpd[b, 4 * g:4 * g + 4] = res1[k]["vpd"]
    cov_nat = cov_dev[:, :, P_INV][:, :, :, P_INV]
    DEBUG["cov_all"] = cov_nat
    U_nat = _host_eigh(cov_nat.reshape(-1, 256, 256))   # (32, 256, 100)
    U_dev = U_nat[:, P_NAT, :].reshape(B, C, 256, TOP_P)

    # ---- L2 (+ folded projection partials)
    nc2 = _get("p2")
    wfrT = np.ascontiguousarray(w_fr.T)
    in2 = []
    for k in range(NCORES):
        b, g = divmod(k, 4)
        ut = np.ascontiguousarray(
            U_dev[b, 4 * g:4 * g + 4].transpose(2, 0, 1).reshape(100, 1024)
        ).astype(np.float32)
        wpoD = np.zeros((32, 128), np.float32)
        for s in range(8):
            for u in range(4):
                wpoD[s * 4 + u, s * 16:s * 16 + 16] = w_po[:, 4 * g + u]
        in2.append({
            "ut": ut,
            "vpd": np.ascontiguousarray(vpd[b, 4 * g:4 * g + 4]),
            "wfrT": wfrT,
            "wpoD": wpoD.astype(bf16),
        })
    res2 = _run("p2", nc2, in2)

    # ---- host: sum partials, unpermute np, second patchify (P2)
    out = np.zeros((B, C, 256, 256), np.float32)
    for b in range(B):
        acc = np.zeros((128, 8192), np.float32)
        for g in range(4):
            acc += res2[4 * b + g]["po"].astype(np.float32)
        # partitions (sp8, o16); free (pl16, mc2, d256);
        # n_dev = mc*128 + sp*16 + pl
        m_dev = acc.reshape(8, 16, 16, 2, 256).transpose(1, 3, 0, 2, 4)
        m_dev = m_dev.reshape(16, 256, 256)
        m_nat = np.zeros_like(m_dev)
        m_nat[:, P_NAT, :] = m_dev
        DEBUG.setdefault("mperm", {})[b] = m_nat
        # P2: patchify the (np, d) matrix per channel
        out[b] = m_nat.reshape(16, 16, 16, 16, 16).transpose(
            0, 1, 3, 2, 4).reshape(16, 256, 256)
    DEBUG["mperm"] = out.copy()
    return out
